# revision 8
# baseline (speedup 1.0000x reference)
"""Trainium2 Bass kernel for a GCN layer (nn_GCNLayer_5669356831736).

Math (per batch b, N=256 nodes, D=K=128):
    xi = x @ w_vi ; xj = x @ w_vj
    H[i,j,:]   = relu(xi[i,:] + xj[j,:] + alpha[i,j,:] + bias_h)
    AH[i,:]    = sum_j adj[i,j] * H[i,j,:]
    new_x      = relu(AH @ w_node)
    new_alpha  = relu(H @ w_alpha)

Sharding: data-parallel over B=8 across the 8 NeuronCores (one batch per
core, SPMD - same NEFF, per-core input slices).

Per-core dataflow (memory-bound target; alpha+new_alpha are ~67MB of HBM
traffic per core):
  - alpha streamed in 2MB slabs laid out [j-part(128), (i, jc, d)] so every
    DMA piece is a contiguous 512B d-row.
  - The outer-sum xi[i]+xj[j]+bias is injected into PSUM by TensorE matmuls
    (identity / ones-broadcast tricks) in float32r (full-rate fp32 on PE for
    moving free dim >= 256), accumulated on top of an identity-matmul copy of
    the alpha chunk.  H = relu(PSUM) is then a single DVE op -> bf16 SBUF.
  - AH:   per (i, jc) tile one n=1 matmul H_tile^T @ adj_col accumulated in a
          persistent PSUM tile AH^T[d, i].
  - new_alpha: PE transpose of each H tile (bf16, identity moving operand) ->
          PSUM -> DVE copy to SBUF -> matmul with w_alpha -> PSUM -> ACT relu
          into a staging slab -> 2MB store.
"""

import os
import numpy as np

import concourse.bass as bass
import concourse.bacc as bacc
import concourse.mybir as mybir
from concourse import masks
from concourse.tile import TileContext
from concourse.bass_utils import run_bass_kernel_spmd

F32 = mybir.dt.float32
F32R = mybir.dt.float32r
BF16 = mybir.dt.bfloat16
AF = mybir.ActivationFunctionType

B, N, D, K = 8, 256, 128, 128
P = 128                 # partitions / j-chunk size
NJC = N // P            # j chunks per i (2)
SLAB_I = 16             # i rows per DMA slab (2MB fp32 HBM reads)
CHUNK_I = 2             # i rows per PSUM chunk -> [128, CHUNK_I*NJC*D] = [128,512]
CHUNK_COLS = CHUNK_I * NJC * D

# Which engine does the HT psum->sbuf copy per chunk index (load balance)
HT_COPY_DVE_OF = (7, 7)   # chunk_no % [1] < [0] -> DVE else ACT


def _build_kernel():
    nc = bacc.Bacc("TRN2", target_bir_lowering=False, debug=False, num_devices=B)

    x_d = nc.dram_tensor("x", [N, D], F32, kind="ExternalInput").ap()
    alpha_d = nc.dram_tensor("alpha", [N, N, D], F32, kind="ExternalInput").ap()
    adj_d = nc.dram_tensor("adj", [N, N], F32, kind="ExternalInput").ap()
    wvi_d = nc.dram_tensor("w_vi", [D, D], F32, kind="ExternalInput").ap()
    wvj_d = nc.dram_tensor("w_vj", [D, D], F32, kind="ExternalInput").ap()
    bias_d = nc.dram_tensor("bias_h", [D], F32, kind="ExternalInput").ap()
    wnode_d = nc.dram_tensor("w_node", [D, K], F32, kind="ExternalInput").ap()
    walpha_d = nc.dram_tensor("w_alpha", [D, K], F32, kind="ExternalInput").ap()

    newx_d = nc.dram_tensor("new_x", [N, K], F32, kind="ExternalOutput").ap()
    newalpha_d = nc.dram_tensor("new_alpha", [N, N, K], F32, kind="ExternalOutput").ap()

    with TileContext(nc) as tc:
        _body(tc, x_d, alpha_d, adj_d, wvi_d, wvj_d, bias_d, wnode_d, walpha_d,
              newx_d, newalpha_d)
    nc.compile()
    return nc


def _body(tc, x_d, alpha_d, adj_d, wvi_d, wvj_d, bias_d, wnode_d, walpha_d,
          newx_d, newalpha_d):
    from contextlib import ExitStack
    nc = tc.nc
    ctx = ExitStack()
    with ctx:
        const = ctx.enter_context(tc.tile_pool(name="const", bufs=1))

        # --- constants (bf16 everywhere the PE touches) --------------------
        ident_bf16 = const.tile([P, P], BF16)
        masks.make_identity(nc, ident_bf16[:])
        ones_bf16 = const.tile([1, P], BF16)
        nc.gpsimd.memset(ones_bf16[:], 1.0)

        wvi = const.tile([D, D], BF16)
        nc.gpsimd.dma_start(wvi[:], wvi_d)
        wvj = const.tile([D, D], BF16)
        nc.gpsimd.dma_start(wvj[:], wvj_d)
        wnode = const.tile([D, K], F32)
        nc.sync.dma_start(wnode[:], wnode_d)
        walpha = const.tile([D, K], BF16)
        nc.gpsimd.dma_start(walpha[:], walpha_d)
        brow = const.tile([1, D], BF16)
        nc.gpsimd.dma_start(brow[:], bias_d.unsqueeze(0))

        xt = [const.tile([P, D], BF16, name=f"xt{c}") for c in range(2)]
        nc.gpsimd.dma_start(xt[0][:], x_d[0:P, :])
        nc.gpsimd.dma_start(xt[1][:], x_d[P:N, :])
        adjt_in = [const.tile([P, N], BF16, name=f"adjt_in{c}") for c in range(2)]
        nc.gpsimd.dma_start(adjt_in[0][:], adj_d[0:P, :])
        nc.gpsimd.dma_start(adjt_in[1][:], adj_d[P:N, :])

        xT = const.tile([D, N], BF16)          # x transposed [d, n]
        xib = [const.tile([P, D], BF16, name=f"xib{c}") for c in range(2)]
        xj = [const.tile([P, D], BF16, name=f"xj{c}") for c in range(2)]
        adjT = [const.tile([P, N], BF16, name=f"adjT{c}") for c in range(2)]

        a_pool = ctx.enter_context(tc.tile_pool(name="alpha", bufs=2))
        o_pool = ctx.enter_context(tc.tile_pool(name="out", bufs=2))
        rows_pool = ctx.enter_context(tc.tile_pool(name="rows", bufs=2))
        h_pool = ctx.enter_context(tc.tile_pool(name="h", bufs=3))
        hts_pool = ctx.enter_context(tc.tile_pool(name="hts", bufs=3))
        s_psum = ctx.enter_context(tc.tile_pool(name="s_psum", bufs=2, space="PSUM"))
        ht_psum = ctx.enter_context(tc.tile_pool(name="ht_psum", bufs=2, space="PSUM"))
        na_psum = ctx.enter_context(tc.tile_pool(name="na_psum", bufs=2, space="PSUM"))
        ah_pool = ctx.enter_context(tc.tile_pool(name="ah", bufs=1, space="PSUM"))

        def pre_tile_bf16():
            return s_psum.tile([P, CHUNK_COLS], BF16, name="Sb", tag="S")

        def pre_tile_f32():
            return s_psum.tile([P, CHUNK_COLS], F32, name="S", tag="S")

        # transpose x -> xT
        for c in range(2):
            pt = pre_tile_bf16()
            nc.tensor.transpose(pt[:, 0:P], xt[c][:], ident_bf16[:])
            nc.vector.tensor_copy(xT[:, c * P:(c + 1) * P], pt[:, 0:P])
        # xi + bias and xj
        for c in range(2):
            pi = pre_tile_f32()
            nc.tensor.matmul(pi[:, 0:D], xT[:, c * P:(c + 1) * P], wvi[:],
                             start=True, stop=False, skip_group_check=True)
            nc.tensor.matmul(pi[:, 0:D], ones_bf16[:], brow[:],
                             start=False, stop=True, skip_group_check=True)
            nc.vector.tensor_copy(xib[c][:], pi[:, 0:D])
            pj = pre_tile_f32()
            nc.tensor.matmul(pj[:, 0:D], xT[:, c * P:(c + 1) * P], wvj[:],
                             start=True, stop=True)
            nc.vector.tensor_copy(xj[c][:], pj[:, 0:D])
        # adj transpose (per jc block col)
        for jc in range(2):
            for ic in range(2):
                pa = pre_tile_bf16()
                nc.tensor.transpose(pa[:, 0:P], adjt_in[ic][:, jc * P:(jc + 1) * P],
                                    ident_bf16[:])
                nc.vector.tensor_copy(adjT[jc][:, ic * P:(ic + 1) * P], pa[:, 0:P])

        # --- main loop -----------------------------------------------------
        ident_r = ident_bf16[:]
        ones_r = ones_bf16[:]

        AH = ah_pool.tile([D, N], F32)   # AH^T accumulator [d, i]
        slab_cols = SLAB_I * NJC * D
        n_chunks = SLAB_I // CHUNK_I

        chunk_no = 0
        for s in range(N // SLAB_I):
            i0 = s * SLAB_I
            aslab = a_pool.tile([P, slab_cols], BF16)
            nc.gpsimd.dma_start(
                aslab[:],
                alpha_d[i0:i0 + SLAB_I].rearrange("i (c p) d -> p i c d", p=P))
            # xib rows of this slab gathered onto partition 0: [1, SLAB_I*D]
            rowslab = rows_pool.tile([1, SLAB_I * D], BF16)
            ci = i0 // P
            r0 = i0 % P
            nc.sync.dma_start(rowslab[:], xib[ci][r0:r0 + SLAB_I, :])
            oslab = o_pool.tile([P, slab_cols], F32)

            for cc in range(n_chunks):
                base = cc * CHUNK_COLS
                S = s_psum.tile([P, CHUNK_COLS], F32, name="S", tag="S")
                # alpha chunk -> PSUM (identity matmul, f32r full rate)
                nc.tensor.matmul(
                    S[:], ident_r,
                    aslab[:, base:base + CHUNK_COLS],
                    start=True, stop=False, skip_group_check=True)
                # + xj[jc] broadcast over the CHUNK_I i-blocks
                Sv = S[:].rearrange("p (i c d) -> p i c d", i=CHUNK_I, c=NJC)
                for jc in range(NJC):
                    rhs = (xj[jc][:, :].unsqueeze(1)
                           .broadcast_to([P, CHUNK_I, D]))
                    nc.tensor.matmul(Sv[:, :, jc, :], ident_r, rhs,
                                     start=False, stop=False,
                                     skip_group_check=True)
                # + (xi[i]+bias) broadcast over partitions and jc
                for ii in range(CHUNK_I):
                    li = cc * CHUNK_I + ii
                    rhs = (rowslab[0:1, li * D:(li + 1) * D].unsqueeze(1)
                           .broadcast_to([1, NJC, D]))
                    nc.tensor.matmul(S[:, ii * NJC * D:(ii + 1) * NJC * D],
                                     ones_r, rhs,
                                     start=False, stop=(ii == CHUNK_I - 1),
                                     skip_group_check=True)
                # H = relu(S) -> bf16
                H = h_pool.tile([P, CHUNK_COLS], BF16)
                nc.vector.tensor_scalar_max(H[:], S[:], 0.0)

                # per tile: transpose H and accumulate AH
                HTp = ht_psum.tile([P, CHUNK_COLS], BF16)
                for t in range(CHUNK_I * NJC):
                    ii, jc = divmod(t, NJC)
                    i = i0 + cc * CHUNK_I + ii
                    Ht = H[:, t * D:(t + 1) * D]
                    nc.tensor.transpose(HTp[:, t * P:(t + 1) * P], Ht,
                                        ident_bf16[:])
                    nc.tensor.matmul(AH[:, i:i + 1], Ht, adjT[jc][:, i:i + 1],
                                     start=(jc == 0), stop=(jc == NJC - 1),
                                     skip_group_check=True)
                HTs = hts_pool.tile([P, CHUNK_COLS], BF16)
                if chunk_no % HT_COPY_DVE_OF[1] < HT_COPY_DVE_OF[0]:
                    nc.vector.tensor_copy(HTs[:], HTp[:])
                else:
                    nc.scalar.copy(HTs[:], HTp[:])

                # new_alpha = relu(H @ w_alpha)
                NAp = na_psum.tile([P, CHUNK_COLS], F32, name="NAp", tag="NAp")
                for t in range(CHUNK_I * NJC):
                    nc.tensor.matmul(NAp[:, t * K:(t + 1) * K],
                                     HTs[:, t * P:(t + 1) * P], walpha[:],
                                     start=True, stop=True)
                nc.scalar.activation(oslab[:, base:base + CHUNK_COLS], NAp[:],
                                     AF.Relu)
                chunk_no += 1

            nc.scalar.dma_start(
                newalpha_d[i0:i0 + SLAB_I].rearrange("i (c p) k -> p i c k", p=P),
                oslab[:])

        # --- new_x = relu(AH @ w_node) ------------------------------------
        AHs = const.tile([D, N], F32)
        nc.vector.tensor_copy(AHs[:], AH[:])
        for c in range(2):
            nxp = na_psum.tile([P, K], F32, name="nxp", tag="NAp")
            nc.tensor.matmul(nxp[:], AHs[:, c * P:(c + 1) * P], wnode[:],
                             start=True, stop=True)
            nxs = const.tile([P, K], F32)
            nc.scalar.activation(nxs[:], nxp[:], AF.Relu)
            nc.sync.dma_start(newx_d[c * P:(c + 1) * P, :], nxs[:])


_NC_CACHE = None


def _get_nc():
    global _NC_CACHE
    if _NC_CACHE is None:
        _NC_CACHE = _build_kernel()
    return _NC_CACHE


def kernel(x, alpha, adj, box_num, w_vi, w_vj, bias_h, w_node, w_alpha,
           **_unused):
    x = np.ascontiguousarray(np.asarray(x, dtype=np.float32))
    alpha = np.ascontiguousarray(np.asarray(alpha, dtype=np.float32))
    adj = np.ascontiguousarray(np.asarray(adj, dtype=np.float32))
    w_vi = np.ascontiguousarray(np.asarray(w_vi, dtype=np.float32))
    w_vj = np.ascontiguousarray(np.asarray(w_vj, dtype=np.float32))
    bias_h = np.ascontiguousarray(np.asarray(bias_h, dtype=np.float32))
    w_node = np.ascontiguousarray(np.asarray(w_node, dtype=np.float32))
    w_alpha = np.ascontiguousarray(np.asarray(w_alpha, dtype=np.float32))

    nc = _get_nc()
    in_maps = [
        {
            "x": x[b], "alpha": alpha[b], "adj": adj[b],
            "w_vi": w_vi, "w_vj": w_vj, "bias_h": bias_h,
            "w_node": w_node, "w_alpha": w_alpha,
        }
        for b in range(B)
    ]
    res = run_bass_kernel_spmd(
        nc, in_maps, core_ids=list(range(B)),
        trace=bool(int(os.environ.get("KERNEL_TRACE", "0"))),
    )
    new_x = np.stack([r["new_x"] for r in res.results])
    new_alpha = np.stack([r["new_alpha"] for r in res.results])
    kernel.last_results = res
    return new_x, new_alpha


# revision 13
# speedup vs baseline: 581.9013x; 581.9013x over previous
"""Trainium2 Bass kernel for a GCN layer (nn_GCNLayer_5669356831736).

Math (per batch b, N=256 nodes, D=K=128):
    xi = x @ w_vi ; xj = x @ w_vj
    H[i,j,:]   = relu(xi[i,:] + xj[j,:] + alpha[i,j,:] + bias_h)
    AH[i,:]    = sum_j adj[i,j] * H[i,j,:]
    new_x      = relu(AH @ w_node)
    new_alpha  = relu(H @ w_alpha)

Sharding: data-parallel over B=8 across the 8 NeuronCores (one batch per
core, SPMD - same NEFF, per-core input slices).

Per-core dataflow (memory-bound target; alpha+new_alpha are ~67MB of HBM
traffic per core):
  - alpha streamed in 2MB slabs laid out [j-part(128), (i, jc, d)] so every
    DMA piece is a contiguous 512B d-row.
  - The outer-sum xi[i]+xj[j]+bias is injected into PSUM by TensorE matmuls
    (identity / ones-broadcast tricks) in float32r (full-rate fp32 on PE for
    moving free dim >= 256), accumulated on top of an identity-matmul copy of
    the alpha chunk.  H = relu(PSUM) is then a single DVE op -> bf16 SBUF.
  - AH:   per (i, jc) tile one n=1 matmul H_tile^T @ adj_col accumulated in a
          persistent PSUM tile AH^T[d, i].
  - new_alpha: PE transpose of each H tile (bf16, identity moving operand) ->
          PSUM -> DVE copy to SBUF -> matmul with w_alpha -> PSUM -> ACT relu
          into a staging slab -> 2MB store.
"""

import os
import numpy as np

import concourse.bass as bass
import concourse.bacc as bacc
import concourse.mybir as mybir
from concourse import masks
from concourse.tile import TileContext
from concourse.bass_utils import run_bass_kernel_spmd

F32 = mybir.dt.float32
F32R = mybir.dt.float32r
BF16 = mybir.dt.bfloat16
AF = mybir.ActivationFunctionType

B, N, D, K = 8, 256, 128, 128
P = 128                 # partitions / j-chunk size
NJC = N // P            # j chunks per i (2)
SLAB_I = 16             # i rows per DMA slab (2MB fp32 HBM reads)
CHUNK_I = 2             # i rows per PSUM chunk -> [128, CHUNK_I*NJC*D] = [128,512]
CHUNK_COLS = CHUNK_I * NJC * D

# Which engine does the HT psum->sbuf copy per chunk index (load balance)
HT_COPY_DVE_OF = (7, 7)   # chunk_no % [1] < [0] -> DVE else ACT
A_BUFS, O_BUFS, H_BUFS, HTS_BUFS = 3, 3, 4, 4
S_BUFS, HTP_BUFS, NA_BUFS = 2, 2, 2
STORE_SPLIT = 1


def _build_kernel(repeat=1, hw_loop=1):
    nc = bacc.Bacc("TRN2", target_bir_lowering=False, debug=False, num_devices=B)

    x_d = nc.dram_tensor("x", [N, D], F32, kind="ExternalInput").ap()
    alpha_d = nc.dram_tensor("alpha", [N, N, D], F32, kind="ExternalInput").ap()
    adj_d = nc.dram_tensor("adj", [N, N], F32, kind="ExternalInput").ap()
    wvi_d = nc.dram_tensor("w_vi", [D, D], F32, kind="ExternalInput").ap()
    wvj_d = nc.dram_tensor("w_vj", [D, D], F32, kind="ExternalInput").ap()
    bias_d = nc.dram_tensor("bias_h", [D], F32, kind="ExternalInput").ap()
    wnode_d = nc.dram_tensor("w_node", [D, K], F32, kind="ExternalInput").ap()
    walpha_d = nc.dram_tensor("w_alpha", [D, K], F32, kind="ExternalInput").ap()

    newx_d = nc.dram_tensor("new_x", [N, K], F32, kind="ExternalOutput").ap()
    newalpha_d = nc.dram_tensor("new_alpha", [N, N, K], F32, kind="ExternalOutput").ap()

    with TileContext(nc) as tc:
        _body(tc, x_d, alpha_d, adj_d, wvi_d, wvj_d, bias_d, wnode_d, walpha_d,
              newx_d, newalpha_d, repeat=repeat, hw_loop=hw_loop)
    nc.compile()
    return nc


def _body(tc, x_d, alpha_d, adj_d, wvi_d, wvj_d, bias_d, wnode_d, walpha_d,
          newx_d, newalpha_d, repeat=1, hw_loop=1):
    from contextlib import ExitStack
    nc = tc.nc
    ctx = ExitStack()
    with ctx:
        const = ctx.enter_context(tc.tile_pool(name="const", bufs=1))

        # --- constants (bf16 everywhere the PE touches) --------------------
        ident_bf16 = const.tile([P, P], BF16)
        masks.make_identity(nc, ident_bf16[:])
        ones_bf16 = const.tile([1, P], BF16)
        nc.gpsimd.memset(ones_bf16[:], 1.0)

        wvi = const.tile([D, D], BF16)
        nc.gpsimd.dma_start(wvi[:], wvi_d)
        wvj = const.tile([D, D], BF16)
        nc.gpsimd.dma_start(wvj[:], wvj_d)
        wnode = const.tile([D, K], F32)
        nc.sync.dma_start(wnode[:], wnode_d)
        walpha = const.tile([D, K], BF16)
        nc.gpsimd.dma_start(walpha[:], walpha_d)
        brow = const.tile([1, D], BF16)
        nc.gpsimd.dma_start(brow[:], bias_d.unsqueeze(0))

        xt = [const.tile([P, D], BF16, name=f"xt{c}") for c in range(2)]
        nc.gpsimd.dma_start(xt[0][:], x_d[0:P, :])
        nc.gpsimd.dma_start(xt[1][:], x_d[P:N, :])
        adjt_in = [const.tile([P, N], BF16, name=f"adjt_in{c}") for c in range(2)]
        nc.gpsimd.dma_start(adjt_in[0][:], adj_d[0:P, :])
        nc.gpsimd.dma_start(adjt_in[1][:], adj_d[P:N, :])

        xT = const.tile([D, N], BF16)          # x transposed [d, n]
        xib = [const.tile([P, D], BF16, name=f"xib{c}") for c in range(2)]
        xj = [const.tile([P, D], BF16, name=f"xj{c}") for c in range(2)]
        adjT = [const.tile([P, N], BF16, name=f"adjT{c}") for c in range(2)]

        a_pool = ctx.enter_context(tc.tile_pool(name="alpha", bufs=A_BUFS))
        o_pool = ctx.enter_context(tc.tile_pool(name="out", bufs=O_BUFS))
        rows_pool = ctx.enter_context(tc.tile_pool(name="rows", bufs=2))
        h_pool = ctx.enter_context(tc.tile_pool(name="h", bufs=H_BUFS))
        hts_pool = ctx.enter_context(tc.tile_pool(name="hts", bufs=HTS_BUFS))
        s_psum = ctx.enter_context(tc.tile_pool(name="s_psum", bufs=S_BUFS, space="PSUM"))
        ht_psum = ctx.enter_context(tc.tile_pool(name="ht_psum", bufs=HTP_BUFS, space="PSUM"))
        na_psum = ctx.enter_context(tc.tile_pool(name="na_psum", bufs=NA_BUFS, space="PSUM"))
        ah_pool = ctx.enter_context(tc.tile_pool(name="ah", bufs=1, space="PSUM"))

        def pre_tile_bf16():
            return s_psum.tile([P, CHUNK_COLS], BF16, name="Sb", tag="S")

        def pre_tile_f32():
            return s_psum.tile([P, CHUNK_COLS], F32, name="S", tag="S")

        # transpose x -> xT
        for c in range(2):
            pt = pre_tile_bf16()
            nc.tensor.transpose(pt[:, 0:P], xt[c][:], ident_bf16[:])
            nc.vector.tensor_copy(xT[:, c * P:(c + 1) * P], pt[:, 0:P])
        # xi + bias and xj
        for c in range(2):
            pi = pre_tile_f32()
            nc.tensor.matmul(pi[:, 0:D], xT[:, c * P:(c + 1) * P], wvi[:],
                             start=True, stop=False, skip_group_check=True)
            nc.tensor.matmul(pi[:, 0:D], ones_bf16[:], brow[:],
                             start=False, stop=True, skip_group_check=True)
            nc.vector.tensor_copy(xib[c][:], pi[:, 0:D])
            pj = pre_tile_f32()
            nc.tensor.matmul(pj[:, 0:D], xT[:, c * P:(c + 1) * P], wvj[:],
                             start=True, stop=True)
            nc.vector.tensor_copy(xj[c][:], pj[:, 0:D])
        # adj transpose (per jc block col)
        for jc in range(2):
            for ic in range(2):
                pa = pre_tile_bf16()
                nc.tensor.transpose(pa[:, 0:P], adjt_in[ic][:, jc * P:(jc + 1) * P],
                                    ident_bf16[:])
                nc.vector.tensor_copy(adjT[jc][:, ic * P:(ic + 1) * P], pa[:, 0:P])

        # --- main loop -----------------------------------------------------
        ident_r = ident_bf16[:]
        ones_r = ones_bf16[:]

        AH = ah_pool.tile([D, N], F32)   # AH^T accumulator [d, i]
        slab_cols = SLAB_I * NJC * D
        n_chunks = SLAB_I // CHUNK_I

        chunk_no = 0

        def run_slab(s, chunk_no):
            i0 = s * SLAB_I
            aslab = a_pool.tile([P, slab_cols], BF16)
            nc.gpsimd.dma_start(
                aslab[:],
                alpha_d[i0:i0 + SLAB_I].rearrange("i (c p) d -> p i c d", p=P))
            # xib rows of this slab gathered onto partition 0: [1, SLAB_I*D]
            rowslab = rows_pool.tile([1, SLAB_I * D], BF16)
            ci = i0 // P
            r0 = i0 % P
            nc.sync.dma_start(rowslab[:], xib[ci][r0:r0 + SLAB_I, :])
            oslab = o_pool.tile([P, slab_cols], F32)

            for cc in range(n_chunks):
                base = cc * CHUNK_COLS
                S = s_psum.tile([P, CHUNK_COLS], F32, name="S", tag="S")
                # alpha chunk -> PSUM (identity matmul, f32r full rate)
                nc.tensor.matmul(
                    S[:], ident_r,
                    aslab[:, base:base + CHUNK_COLS],
                    start=True, stop=False, skip_group_check=True)
                # + xj[jc] broadcast over the CHUNK_I i-blocks
                Sv = S[:].rearrange("p (i c d) -> p i c d", i=CHUNK_I, c=NJC)
                for jc in range(NJC):
                    rhs = (xj[jc][:, :].unsqueeze(1)
                           .broadcast_to([P, CHUNK_I, D]))
                    nc.tensor.matmul(Sv[:, :, jc, :], ident_r, rhs,
                                     start=False, stop=False,
                                     skip_group_check=True)
                # + (xi[i]+bias) broadcast over partitions and jc
                for ii in range(CHUNK_I):
                    li = cc * CHUNK_I + ii
                    rhs = (rowslab[0:1, li * D:(li + 1) * D].unsqueeze(1)
                           .broadcast_to([1, NJC, D]))
                    nc.tensor.matmul(S[:, ii * NJC * D:(ii + 1) * NJC * D],
                                     ones_r, rhs,
                                     start=False, stop=(ii == CHUNK_I - 1),
                                     skip_group_check=True)
                # H = relu(S) -> bf16
                H = h_pool.tile([P, CHUNK_COLS], BF16)
                nc.vector.tensor_scalar_max(H[:], S[:], 0.0)

                # per tile: transpose H and accumulate AH
                HTp = ht_psum.tile([P, CHUNK_COLS], BF16)
                for t in range(CHUNK_I * NJC):
                    ii, jc = divmod(t, NJC)
                    i = i0 + cc * CHUNK_I + ii
                    Ht = H[:, t * D:(t + 1) * D]
                    nc.tensor.transpose(HTp[:, t * P:(t + 1) * P], Ht,
                                        ident_bf16[:])
                    nc.tensor.matmul(AH[:, i:i + 1], Ht, adjT[jc][:, i:i + 1],
                                     start=(jc == 0), stop=(jc == NJC - 1),
                                     skip_group_check=True)
                HTs = hts_pool.tile([P, CHUNK_COLS], BF16)
                if chunk_no % HT_COPY_DVE_OF[1] < HT_COPY_DVE_OF[0]:
                    nc.vector.tensor_copy(HTs[:], HTp[:])
                else:
                    nc.scalar.copy(HTs[:], HTp[:])

                # new_alpha = relu(H @ w_alpha)
                NAp = na_psum.tile([P, CHUNK_COLS], F32, name="NAp", tag="NAp")
                for t in range(CHUNK_I * NJC):
                    nc.tensor.matmul(NAp[:, t * K:(t + 1) * K],
                                     HTs[:, t * P:(t + 1) * P], walpha[:],
                                     start=True, stop=True)
                nc.scalar.activation(oslab[:, base:base + CHUNK_COLS], NAp[:],
                                     AF.Relu)
                chunk_no += 1

            for h in range(STORE_SPLIT):
                ih = SLAB_I // STORE_SPLIT
                nc.scalar.dma_start(
                    newalpha_d[i0 + h * ih:i0 + (h + 1) * ih]
                    .rearrange("i (c p) k -> p i c k", p=P),
                    oslab[:, h * ih * NJC * D:(h + 1) * ih * NJC * D])

            return chunk_no

        import contextlib
        if hw_loop > 1:
            loop_ctx = tc.For_i(0, hw_loop, 1, hint_engines=tuple(
                mybir.EngineType(e) for e in ("PE", "DVE", "Activation", "Pool", "SP")))
        else:
            loop_ctx = contextlib.nullcontext()
        with loop_ctx:
            for s in range(repeat * (N // SLAB_I)):
                chunk_no = run_slab(s % (N // SLAB_I), chunk_no)

        # --- new_x = relu(AH @ w_node) ------------------------------------
        AHs = const.tile([D, N], F32)
        nc.vector.tensor_copy(AHs[:], AH[:])
        for c in range(2):
            nxp = na_psum.tile([P, K], F32, name="nxp", tag="NAp")
            nc.tensor.matmul(nxp[:], AHs[:, c * P:(c + 1) * P], wnode[:],
                             start=True, stop=True)
            nxs = const.tile([P, K], F32)
            nc.scalar.activation(nxs[:], nxp[:], AF.Relu)
            nc.sync.dma_start(newx_d[c * P:(c + 1) * P, :], nxs[:])


_NC_CACHE = None


def _get_nc():
    global _NC_CACHE
    if _NC_CACHE is None:
        _NC_CACHE = _build_kernel()
    return _NC_CACHE


def kernel(x, alpha, adj, box_num, w_vi, w_vj, bias_h, w_node, w_alpha,
           **_unused):
    x = np.ascontiguousarray(np.asarray(x, dtype=np.float32))
    alpha = np.ascontiguousarray(np.asarray(alpha, dtype=np.float32))
    adj = np.ascontiguousarray(np.asarray(adj, dtype=np.float32))
    w_vi = np.ascontiguousarray(np.asarray(w_vi, dtype=np.float32))
    w_vj = np.ascontiguousarray(np.asarray(w_vj, dtype=np.float32))
    bias_h = np.ascontiguousarray(np.asarray(bias_h, dtype=np.float32))
    w_node = np.ascontiguousarray(np.asarray(w_node, dtype=np.float32))
    w_alpha = np.ascontiguousarray(np.asarray(w_alpha, dtype=np.float32))

    nc = _get_nc()
    in_maps = [
        {
            "x": x[b], "alpha": alpha[b], "adj": adj[b],
            "w_vi": w_vi, "w_vj": w_vj, "bias_h": bias_h,
            "w_node": w_node, "w_alpha": w_alpha,
        }
        for b in range(B)
    ]
    res = run_bass_kernel_spmd(
        nc, in_maps, core_ids=list(range(B)),
        trace=bool(int(os.environ.get("KERNEL_TRACE", "0"))),
    )
    new_x = np.stack([r["new_x"] for r in res.results])
    new_alpha = np.stack([r["new_alpha"] for r in res.results])
    kernel.last_results = res
    return new_x, new_alpha
